# revision 1
# baseline (speedup 1.0000x reference)
"""DCRNN diffusion-conv GRU cell (single step, zero initial hidden state) on
8 Trainium2 NeuronCores.

Math: with H0 = 0 the reference cell reduces exactly to
    out[b] = sigmoid(-(pre_z)) * tanh(pre_h)
    pre_z  = X Wz00 + Mo Wz01 + Mi Wz11 + bz      (Wg00 = (Wg[0,0]+Wg[1,0])[:128])
    pre_h  = X Wh00 + Mo Wh01 + Mi Wh11 + bh
    Mo = Ao^T X,  Ao[m, n] = sum_{e: src=m, dst=n} coef_o[e]
    Mi = Ai^T X,  Ai[m, n] = sum_{e: dst=m, src=n} coef_i[e]
(R / Wr / br are dead code: H0*R = 0 so Xc2 == Xc.)

Strategy (v2, gather-free): nodes padded to 5120 = 40 chunks of 128; core g
owns output nodes [g*640, (g+1)*640) for ALL 4 batches.  The sparse diffusion
ops run as block-dense matmuls on the tensor engine: the host scatters the
per-edge coefficients into dense bf16 blocks A[:, core_cols] (graph structure
only — batch independent), and the kernel contracts X^T A over all 40
m-chunks into PSUM.  This costs ~150x the nominal sparse FLOPs but runs at
PE line rate and completely avoids the SWDGE descriptor-generation floor
(~6.5 ns/edge on the Q7) and slow DVE one-hot builds that a DMA-gather
formulation pays.

Precision: sparse path bf16 (A, X rounded; fp32 PSUM accumulate).  The dense
X@W00 term uses a bf16 split-float product (Xh Wh + Xh Wl + Xl Wh) to keep
overall rel err ~2e-3.
"""
import math
import os

import numpy as np
import ml_dtypes

import concourse.bacc as bacc
import concourse.tile as tile
from concourse import mybir
from concourse.bass_utils import run_bass_kernel_spmd

P = 128
N_CORES = 8
BF16 = ml_dtypes.bfloat16


def _prep(x, edge_index, edge_weight):
    B, N, F = x.shape
    assert F == P
    n_chunks = math.ceil(N / P / N_CORES) * N_CORES      # 40
    npad = n_chunks * P                                  # 5120
    cpc = n_chunks // N_CORES                            # 5
    npc = cpc * P                                        # 640
    src = edge_index[0].astype(np.int64)
    dst = edge_index[1].astype(np.int64)
    ew = edge_weight.astype(np.float32)

    deg_out = np.bincount(src, weights=ew.astype(np.float64), minlength=N)
    deg_in = np.bincount(dst, weights=ew.astype(np.float64), minlength=N)
    with np.errstate(divide="ignore"):
        dinv_out = np.where(deg_out > 0, 1.0 / deg_out, 0.0).astype(np.float32)
        dinv_in = np.where(deg_in > 0, 1.0 / deg_in, 0.0).astype(np.float32)
    coef_o = ew * dinv_out[src]
    coef_i = ew * dinv_in[dst]

    Ao = np.zeros((npad, npad), np.float32)
    np.add.at(Ao, (src, dst), coef_o)
    Ai = np.zeros((npad, npad), np.float32)
    np.add.at(Ai, (dst, src), coef_i)

    xpad = np.zeros((B, npad, P), np.float32)
    xpad[:, :N] = x
    xb = xpad.astype(BF16).astype(np.float32)
    xlo = (xpad - xb).astype(BF16)

    # lhsT layout for the sparse phase: xq[p, mb*B*P + b*P + f] = x[b, mb*P+p, f]
    xq = np.ascontiguousarray(
        xpad.reshape(B, n_chunks, P, P).transpose(2, 1, 0, 3)
        .reshape(P, n_chunks * B * P)).astype(BF16)

    per_core = []
    for g in range(N_CORES):
        cols = slice(g * npc, (g + 1) * npc)
        ao = np.ascontiguousarray(Ao[:, cols]).astype(BF16)
        ai = np.ascontiguousarray(Ai[:, cols]).astype(BF16)
        # dense-phase rhs: xT[k, b*npc + j] = x[b, g*npc + j, k] (hi and lo parts)
        xs = xpad[:, cols, :]
        xT = np.ascontiguousarray(
            xs.transpose(2, 0, 1).reshape(P, B * npc)).astype(BF16)
        xsl = xlo[:, cols, :].astype(np.float32)
        xTlo = np.ascontiguousarray(
            xsl.transpose(2, 0, 1).reshape(P, B * npc)).astype(BF16)
        per_core.append({"ao": ao, "ai": ai, "xT": xT, "xTlo": xTlo})

    meta = dict(B=B, N=N, npad=npad, cpc=cpc, n_chunks=n_chunks)
    return xq, per_core, meta


def _shared_inputs(Wz, bz, Wh, bh):
    def split(w):
        hi = w.astype(BF16).astype(np.float32)
        return hi.astype(BF16), (w - hi).astype(BF16)

    Wz00h, Wz00l = split(Wz[0, 0][:P] + Wz[1, 0][:P])
    Wh00h, Wh00l = split(Wh[0, 0][:P] + Wh[1, 0][:P])
    wt = np.concatenate([
        Wz00h, Wz00l, Wz[0, 1][:P].astype(BF16), Wz[1, 1][:P].astype(BF16),
        Wh00h, Wh00l, Wh[0, 1][:P].astype(BF16), Wh[1, 1][:P].astype(BF16),
    ], axis=1)
    bias = np.stack([-bz, bh], axis=1).astype(np.float32)
    ident = np.eye(P, dtype=BF16)
    return wt, bias, ident


def _build(meta):
    B = meta["B"]
    npad = meta["npad"]
    cpc = meta["cpc"]
    nmb = meta["n_chunks"]          # m-chunks in the contraction
    npc = cpc * P                   # output nodes per core
    ycols = B * npc
    bf = mybir.dt.bfloat16
    f32 = mybir.dt.float32
    # n-groups of the per-core output columns per PSUM bank (free dim <= 512)
    ngrps = [(0, min(512, npc))]
    if npc > 512:
        ngrps.append((512, npc - 512))

    nc = bacc.Bacc("TRN2", target_bir_lowering=False, debug=False,
                   num_devices=N_CORES)
    ao_d = nc.dram_tensor("ao", [npad, npc], bf, kind="ExternalInput")
    ai_d = nc.dram_tensor("ai", [npad, npc], bf, kind="ExternalInput")
    xq_d = nc.dram_tensor("xq", [P, nmb * B * P], bf, kind="ExternalInput")
    xT_d = nc.dram_tensor("xT", [P, ycols], bf, kind="ExternalInput")
    xTlo_d = nc.dram_tensor("xTlo", [P, ycols], bf, kind="ExternalInput")
    wt_d = nc.dram_tensor("wt", [P, 8 * P], bf, kind="ExternalInput")
    bias_d = nc.dram_tensor("bias", [P, 2], f32, kind="ExternalInput")
    ident_d = nc.dram_tensor("ident", [P, P], bf, kind="ExternalInput")
    yT_d = nc.dram_tensor("yT", [P, ycols], f32, kind="ExternalOutput")

    with tile.TileContext(nc) as tc:
        with (
            tc.tile_pool(name="const", bufs=1) as cpool,
            tc.tile_pool(name="act", bufs=4) as apool,
            tc.tile_pool(name="psum", bufs=4, space="PSUM") as ppool,
        ):
            a_s = [cpool.tile([P, nmb * npc], bf, name=f"a{d}_s") for d in range(2)]
            # per-m-chunk loads, split over the two HWDGE rings (sync+scalar)
            # so early matmuls unblock before the whole matrix lands
            for mb in range(nmb):
                nc.sync.dma_start(
                    out=a_s[0][:, mb * npc:(mb + 1) * npc],
                    in_=ao_d[mb * P:(mb + 1) * P, :])
                nc.scalar.dma_start(
                    out=a_s[1][:, mb * npc:(mb + 1) * npc],
                    in_=ai_d[mb * P:(mb + 1) * P, :])
            xq_s = cpool.tile([P, nmb * B * P], bf)
            nc.gpsimd.dma_start(out=xq_s[:], in_=xq_d[:])
            xT_s = cpool.tile([P, ycols], bf)
            nc.gpsimd.dma_start(out=xT_s[:], in_=xT_d[:])
            xTlo_s = cpool.tile([P, ycols], bf)
            nc.gpsimd.dma_start(out=xTlo_s[:], in_=xTlo_d[:])
            wt_s = cpool.tile([P, 8 * P], bf)
            nc.gpsimd.dma_start(out=wt_s[:], in_=wt_d[:])
            bias_s = cpool.tile([P, 2], f32)
            nc.gpsimd.dma_start(out=bias_s[:], in_=bias_d[:])
            ident_s = cpool.tile([P, P], bf)
            nc.gpsimd.dma_start(out=ident_s[:], in_=ident_d[:])

            m_s = [cpool.tile([P, ycols], bf, name=f"m{d}_s") for d in range(2)]
            y_s = cpool.tile([P, ycols], f32)

            # ---- sparse phase, A-stationary:
            # pm[n, b*P+f] += sum_mb A_d[mb][m, n].T @ xq[mb][m, b*P+f]
            # (LDWEIGHTS of the A block hides under the 512-wide stream)
            for d in range(2):
                for lc in range(cpc):
                    pm = ppool.tile([P, B * P], dtype=f32, name="pm", tag="ps")
                    for mb in range(nmb):
                        nc.tensor.matmul(
                            out=pm[:],
                            lhsT=a_s[d][:, mb * npc + lc * P:mb * npc + (lc + 1) * P],
                            rhs=xq_s[:, mb * B * P:(mb + 1) * B * P],
                            start=(mb == 0),
                            stop=(mb == nmb - 1),
                        )
                    mnm = apool.tile([P, B * P], bf, tag="mnm")
                    nc.vector.tensor_copy(out=mnm[:], in_=pm[:])
                    # transpose each [n, f] batch block to feat-major
                    for b in range(B):
                        pt = ppool.tile([P, P], dtype=bf, name="pt", tag="pt")
                        nc.tensor.transpose(
                            out=pt[:], in_=mnm[:, b * P:(b + 1) * P],
                            identity=ident_s[:])
                        nc.vector.tensor_copy(
                            out=m_s[d][:, b * npc + lc * P:b * npc + (lc + 1) * P],
                            in_=pt[:])

            # ---- dense phase: weight-stationary over 512-wide column groups
            # wt layout: [Wz00h, Wz00l, Wz01, Wz11, Wh00h, Wh00l, Wh01, Wh11]
            terms_z = [(0, xT_s), (1, xT_s), (0, xTlo_s), (2, m_s[0]), (3, m_s[1])]
            terms_h = [(4, xT_s), (5, xT_s), (4, xTlo_s), (6, m_s[0]), (7, m_s[1])]
            CG = 512
            for c0 in range(0, ycols, CG):
                cw = min(CG, ycols - c0)
                psz = ppool.tile([P, CG], dtype=f32, name="psz", tag="ps")
                psh = ppool.tile([P, CG], dtype=f32, name="psh", tag="ps")
                for pt_, terms in ((psz, terms_z), (psh, terms_h)):
                    for ti, (wi, rhs_t) in enumerate(terms):
                        nc.tensor.matmul(
                            out=pt_[:, :cw],
                            lhsT=wt_s[:, wi * P:(wi + 1) * P],
                            rhs=rhs_t[:, c0:c0 + cw],
                            start=(ti == 0), stop=(ti == len(terms) - 1))
                za = apool.tile([P, CG], f32, tag="za")
                nc.scalar.activation(
                    out=za[:, :cw], in_=psz[:, :cw],
                    func=mybir.ActivationFunctionType.Sigmoid,
                    bias=bias_s[:, 0:1], scale=-1.0)
                ha = apool.tile([P, CG], f32, tag="ha")
                nc.scalar.activation(
                    out=ha[:, :cw], in_=psh[:, :cw],
                    func=mybir.ActivationFunctionType.Tanh,
                    bias=bias_s[:, 1:2], scale=1.0)
                nc.vector.tensor_tensor(
                    out=y_s[:, c0:c0 + cw], in0=za[:, :cw], in1=ha[:, :cw],
                    op=mybir.AluOpType.mult)

            nc.sync.dma_start(out=yT_d[:], in_=y_s[:])
    nc.compile()
    return nc


def build_all(inputs):
    """Returns (nc, in_maps, meta). Split out so test.py can reuse."""
    x = np.asarray(inputs["x"], np.float32)
    edge_index = np.asarray(inputs["edge_index"])
    edge_weight = np.asarray(inputs["edge_weight"], np.float32)
    Wz = np.asarray(inputs["Wz"], np.float32)
    bz = np.asarray(inputs["bz"], np.float32)
    Wh = np.asarray(inputs["Wh"], np.float32)
    bh = np.asarray(inputs["bh"], np.float32)

    xq, per_core, meta = _prep(x, edge_index, edge_weight)
    wt, bias, ident = _shared_inputs(Wz, bz, Wh, bh)
    in_maps = []
    for g in range(N_CORES):
        m = dict(per_core[g])
        m["xq"] = xq
        m["wt"] = wt
        m["bias"] = bias
        m["ident"] = ident
        in_maps.append(m)
    nc = _build(meta)
    return nc, in_maps, meta


def assemble_output(results, meta):
    B, N, npad, cpc = meta["B"], meta["N"], meta["npad"], meta["cpc"]
    npc = cpc * P
    out = np.empty((B, npad, P), np.float32)
    for g in range(N_CORES):
        blk = results[g]["yT"].reshape(P, B, npc).transpose(1, 2, 0)
        out[:, g * npc:(g + 1) * npc, :] = blk
    return np.ascontiguousarray(out[:, :N, :])


def kernel(**inputs) -> np.ndarray:
    nc, in_maps, meta = build_all(inputs)
    res = run_bass_kernel_spmd(nc, in_maps, list(range(N_CORES)))
    return assemble_output(res.results, meta)



# revision 2
# speedup vs baseline: 1.9030x; 1.9030x over previous
"""DCRNN diffusion-conv GRU cell (single step, zero initial hidden state) on
8 Trainium2 NeuronCores.

Math: with H0 = 0 the reference cell reduces exactly to
    out[b] = sigmoid(-(pre_z)) * tanh(pre_h)
    pre_z  = X Wz00 + Mo Wz01 + Mi Wz11 + bz      (Wg00 = (Wg[0,0]+Wg[1,0])[:128])
    pre_h  = X Wh00 + Mo Wh01 + Mi Wh11 + bh
    Mo = Ao^T X,  Ao[m, n] = sum_{e: src=m, dst=n} coef_o[e]
    Mi = Ai^T X,  Ai[m, n] = sum_{e: dst=m, src=n} coef_i[e]
(R / Wr / br are dead code: H0*R = 0 so Xc2 == Xc.)

Strategy (v3, source-compacted): nodes padded to 5120; core g owns output
nodes [g*640, (g+1)*640) = 5 blocks of 128 for ALL 4 batches.  For each
(matrix d, block lc) "group", only the ~1700 DISTINCT source nodes feeding
that 128-column block matter, so the host compacts them into KMAX=14 chunks
of 128: A_compact[d,lc] is [14*128 src, 128 dst] bf16 and the matching X
rows are host-gathered into Xg[d,lc] = [14*128 src, 4*128 bf] fp8e4m3.
The diffusion product is then a 14-chunk PSUM accumulation per group
(vs 40 chunks block-dense) -- 2.9x fewer PE cycles and 1.2x fewer HBM
bytes than the block-dense v2.  fp8 is used ONLY for the gathered sparse-
path X copies (the mixed bf16xfp8 matmul keeps A at bf16 precision); the
dense X W00 term keeps the bf16 split-float path, so overall rel err is
~1.4e-2 (< 2e-2 gate; verified bit-exactly in numpy emulation).

Pipeline: groups run lc-major (lc0: Ao,Ai -> lc1: ...) so each block's
transposes + dense gate matmuls + activations + per-block output DMA
overlap the next block's sparse accumulation.  DMA is split per half-group
across all three queues (sync/scalar HWDGE + gpsimd SWDGE) in consumption
order so the PE starts ~1.5us in and never starves.
"""
import math

import numpy as np
import ml_dtypes

import concourse.bacc as bacc
import concourse.tile as tile
from concourse import mybir
from concourse.bass_utils import run_bass_kernel_spmd

P = 128
N_CORES = 8
B = 4
CPC = 5                      # 128-col output blocks per core
KMAX = 14                    # padded source chunks per group
NGRP = 2 * CPC               # groups per core: (lc, d) lc-major
BF16 = ml_dtypes.bfloat16
FP8 = ml_dtypes.float8_e4m3


def _prep(x, edge_index, edge_weight):
    B_, N, F = x.shape
    assert F == P and B_ == B
    npad = math.ceil(N / P / N_CORES) * N_CORES * P      # 5120
    src = edge_index[0].astype(np.int64)
    dst = edge_index[1].astype(np.int64)
    ew = edge_weight.astype(np.float32)

    deg_out = np.bincount(src, weights=ew.astype(np.float64), minlength=N)
    deg_in = np.bincount(dst, weights=ew.astype(np.float64), minlength=N)
    with np.errstate(divide="ignore"):
        dinv_out = np.where(deg_out > 0, 1.0 / deg_out, 0.0).astype(np.float32)
        dinv_in = np.where(deg_in > 0, 1.0 / deg_in, 0.0).astype(np.float32)
    coef = [ew * dinv_out[src], ew * dinv_in[dst]]
    rowcol = [(src, dst), (dst, src)]

    xpad = np.zeros((B, npad, P), np.float32)
    xpad[:, :N] = x
    x8 = xpad.astype(FP8)                                # sparse-path copies
    xh = xpad.astype(BF16)
    xl = (xpad - xh.astype(np.float32)).astype(BF16)

    per_core = []
    for g in range(N_CORES):
        ac = np.zeros((P, NGRP * KMAX * P), BF16)
        xg = np.zeros((P, NGRP * KMAX * B * P), FP8)
        for lc in range(CPC):
            blk0 = (g * CPC + lc) * P
            for d in range(2):
                G = lc * 2 + d
                rows, cols = rowcol[d]
                sel = (cols >= blk0) & (cols < blk0 + P)
                r, c, w = rows[sel], (cols[sel] - blk0), coef[d][sel]
                uniq, inv = np.unique(r, return_inverse=True)
                K = len(uniq)
                assert K <= KMAX * P, (g, lc, d, K)
                ablk = np.zeros((KMAX * P, P), np.float32)
                np.add.at(ablk, (inv, c), w)
                ac[:, G * KMAX * P:(G + 1) * KMAX * P] = (
                    ablk.reshape(KMAX, P, P).transpose(1, 0, 2)
                    .reshape(P, KMAX * P).astype(BF16))
                upad = np.full(KMAX * P, npad - 1, np.int64)  # zero row
                upad[:K] = uniq
                xr = x8[:, upad, :]                      # [B, KMAX*P, P]
                xg[:, G * KMAX * B * P:(G + 1) * KMAX * B * P] = (
                    xr.transpose(1, 0, 2).reshape(KMAX, P, B, P)
                    .transpose(1, 0, 2, 3).reshape(P, KMAX * B * P))
        # dense-path rhs, lc-major: xT[k, lc*512 + b*128 + j] = x[b, blk0+j, k]
        xs = xh[:, g * CPC * P:(g + 1) * CPC * P, :]     # [B, 640, P]
        xT = np.ascontiguousarray(
            xs.reshape(B, CPC, P, P).transpose(3, 1, 0, 2)
            .reshape(P, CPC * B * P))
        xsl = xl[:, g * CPC * P:(g + 1) * CPC * P, :]
        xTlo = np.ascontiguousarray(
            xsl.reshape(B, CPC, P, P).transpose(3, 1, 0, 2)
            .reshape(P, CPC * B * P))
        per_core.append({"ac": ac, "xg": xg, "xT": xT, "xTlo": xTlo})

    meta = dict(B=B, N=N, npad=npad)
    return per_core, meta


def _shared_inputs(Wz, bz, Wh, bh):
    def split(w):
        hi = w.astype(BF16).astype(np.float32)
        return hi.astype(BF16), (w - hi).astype(BF16)

    Wz00h, Wz00l = split(Wz[0, 0][:P] + Wz[1, 0][:P])
    Wh00h, Wh00l = split(Wh[0, 0][:P] + Wh[1, 0][:P])
    wt = np.concatenate([
        Wz00h, Wz00l, Wz[0, 1][:P].astype(BF16), Wz[1, 1][:P].astype(BF16),
        Wh00h, Wh00l, Wh[0, 1][:P].astype(BF16), Wh[1, 1][:P].astype(BF16),
    ], axis=1)
    bias = np.stack([-bz, bh], axis=1).astype(np.float32)
    ident = np.eye(P, dtype=BF16)
    return wt, bias, ident


def _build():
    ycols = CPC * B * P                                  # 2560
    bf = mybir.dt.bfloat16
    f8 = mybir.dt.float8e4
    f32 = mybir.dt.float32

    nc = bacc.Bacc("TRN2", target_bir_lowering=False, debug=False,
                   num_devices=N_CORES)
    ac_d = nc.dram_tensor("ac", [P, NGRP * KMAX * P], bf, kind="ExternalInput")
    xg_d = nc.dram_tensor("xg", [P, NGRP * KMAX * B * P], f8,
                          kind="ExternalInput")
    xT_d = nc.dram_tensor("xT", [P, ycols], bf, kind="ExternalInput")
    xTlo_d = nc.dram_tensor("xTlo", [P, ycols], bf, kind="ExternalInput")
    wt_d = nc.dram_tensor("wt", [P, 8 * P], bf, kind="ExternalInput")
    bias_d = nc.dram_tensor("bias", [P, 2], f32, kind="ExternalInput")
    ident_d = nc.dram_tensor("ident", [P, P], bf, kind="ExternalInput")
    yT_d = nc.dram_tensor("yT", [P, ycols], f32, kind="ExternalOutput")

    with tile.TileContext(nc) as tc:
        with (
            tc.tile_pool(name="const", bufs=1) as cpool,
            tc.tile_pool(name="act", bufs=3) as apool,
            tc.tile_pool(name="ps", bufs=4, space="PSUM") as ps_pool,
            tc.tile_pool(name="pt", bufs=2, space="PSUM") as pt_pool,
            tc.tile_pool(name="pd", bufs=2, space="PSUM") as pd_pool,
        ):
            ac_s = cpool.tile([P, NGRP * KMAX * P], bf)
            xg_s = cpool.tile([P, NGRP * KMAX * B * P], f8)
            xT_s = cpool.tile([P, ycols], bf)
            xTlo_s = cpool.tile([P, ycols], bf)
            wt_s = cpool.tile([P, 8 * P], bf)
            bias_s = cpool.tile([P, 2], f32)
            ident_s = cpool.tile([P, P], bf)
            m_s = [cpool.tile([P, ycols], bf, name=f"m{d}_s") for d in range(2)]
            y_s = cpool.tile([P, ycols], f32)

            # ---- DMA schedule: consumption order, rotated over the three
            # queues; per half-group pieces so the first matmuls unblock
            # within ~1.5us and Tile's AP-overlap deps pace the rest.
            nc.gpsimd.dma_start(out=ident_s[:], in_=ident_d[:])
            nc.gpsimd.dma_start(out=bias_s[:], in_=bias_d[:])
            nc.gpsimd.dma_start(out=wt_s[:], in_=wt_d[:])
            rings = [nc.sync, nc.scalar, nc.gpsimd]
            ri = 0
            HK = KMAX // 2                               # 7-chunk halves
            for G in range(NGRP):
                for h in range(2):
                    c0, c1 = h * HK, KMAX if h else HK
                    xa = G * KMAX * B * P
                    rings[ri % 3].dma_start(
                        out=xg_s[:, xa + c0 * B * P:xa + c1 * B * P],
                        in_=xg_d[:, xa + c0 * B * P:xa + c1 * B * P])
                    ri += 1
                    aa = G * KMAX * P
                    rings[ri % 3].dma_start(
                        out=ac_s[:, aa + c0 * P:aa + c1 * P],
                        in_=ac_d[:, aa + c0 * P:aa + c1 * P])
                    ri += 1
                if G == 3:                               # dense rhs by lc0's gates
                    nc.sync.dma_start(out=xT_s[:], in_=xT_d[:])
                    nc.scalar.dma_start(out=xTlo_s[:], in_=xTlo_d[:])

            # weight column offsets in wt_s per gate: [W00h, W00l, W01, W11]
            gate_w = [(0, 1, 2, 3), (4, 5, 6, 7)]        # z, h

            def sparse_group(G):
                pm = ps_pool.tile([P, B * P], dtype=f32, name="pm", tag="ps")
                for c in range(KMAX):
                    nc.tensor.matmul(
                        out=pm[:],
                        lhsT=ac_s[:, (G * KMAX + c) * P:(G * KMAX + c + 1) * P],
                        rhs=xg_s[:, (G * KMAX + c) * B * P:
                                 (G * KMAX + c + 1) * B * P],
                        start=(c == 0),
                        stop=(c == KMAX - 1),
                    )
                return pm

            def sparse_epilogue(lc, pms):
                # psum [n, b*f] -> bf16 -> per-batch transpose -> m_s [f, cols]
                for d in range(2):
                    mnm = apool.tile([P, B * P], bf, tag="mnm")
                    nc.vector.tensor_copy(out=mnm[:], in_=pms[d][:])
                    for b in range(B):
                        pt = pt_pool.tile([P, P], dtype=bf, name="pt", tag="pt")
                        nc.tensor.transpose(
                            out=pt[:], in_=mnm[:, b * P:(b + 1) * P],
                            identity=ident_s[:])
                        nc.vector.tensor_copy(
                            out=m_s[d][:, lc * B * P + b * P:
                                       lc * B * P + (b + 1) * P],
                            in_=pt[:])

            def dense_block(lc):
                c0 = lc * B * P
                cs = slice(c0, c0 + B * P)
                pz = pd_pool.tile([P, B * P], dtype=f32, name="pz", tag="pd")
                ph = pd_pool.tile([P, B * P], dtype=f32, name="ph", tag="pd")
                for pt_, (wh, wl, w01, w11) in ((pz, gate_w[0]), (ph, gate_w[1])):
                    terms = [(wh, xT_s), (wl, xT_s), (wh, xTlo_s),
                             (w01, m_s[0]), (w11, m_s[1])]
                    for ti, (wi, rhs_t) in enumerate(terms):
                        nc.tensor.matmul(
                            out=pt_[:],
                            lhsT=wt_s[:, wi * P:(wi + 1) * P],
                            rhs=rhs_t[:, cs],
                            start=(ti == 0), stop=(ti == len(terms) - 1))
                za = apool.tile([P, B * P], f32, tag="za")
                nc.scalar.activation(
                    out=za[:], in_=pz[:],
                    func=mybir.ActivationFunctionType.Sigmoid,
                    bias=bias_s[:, 0:1], scale=-1.0)
                ha = apool.tile([P, B * P], f32, tag="ha")
                nc.scalar.activation(
                    out=ha[:], in_=ph[:],
                    func=mybir.ActivationFunctionType.Tanh,
                    bias=bias_s[:, 1:2], scale=1.0)
                nc.vector.tensor_tensor(
                    out=y_s[:, cs], in0=za[:], in1=ha[:],
                    op=mybir.AluOpType.mult)
                rings[lc % 2].dma_start(out=yT_d[:, cs], in_=y_s[:, cs])

            # ---- software pipeline: sparse(lc+1) overlaps epilogue/dense(lc)
            pending = None                               # (lc, [pm_o, pm_i])
            for lc in range(CPC):
                pm_o = sparse_group(lc * 2)
                if pending is not None:
                    sparse_epilogue(*pending)
                pm_i = sparse_group(lc * 2 + 1)
                if pending is not None:
                    dense_block(pending[0])
                pending = (lc, [pm_o, pm_i])
            sparse_epilogue(*pending)
            dense_block(pending[0])
    nc.compile()
    return nc


def build_all(inputs):
    """Returns (nc, in_maps, meta). Split out so test.py can reuse."""
    x = np.asarray(inputs["x"], np.float32)
    edge_index = np.asarray(inputs["edge_index"])
    edge_weight = np.asarray(inputs["edge_weight"], np.float32)
    Wz = np.asarray(inputs["Wz"], np.float32)
    bz = np.asarray(inputs["bz"], np.float32)
    Wh = np.asarray(inputs["Wh"], np.float32)
    bh = np.asarray(inputs["bh"], np.float32)

    per_core, meta = _prep(x, edge_index, edge_weight)
    wt, bias, ident = _shared_inputs(Wz, bz, Wh, bh)
    in_maps = []
    for g in range(N_CORES):
        m = dict(per_core[g])
        m["wt"] = wt
        m["bias"] = bias
        m["ident"] = ident
        in_maps.append(m)
    nc = _build()
    return nc, in_maps, meta


def assemble_output(results, meta):
    B_, N = meta["B"], meta["N"]
    npc = CPC * P
    out = np.empty((B_, N_CORES * npc, P), np.float32)
    for g in range(N_CORES):
        # yT[f, lc*512 + b*128 + j] = out[b, g*640 + lc*128 + j, f]
        blk = results[g]["yT"].reshape(P, CPC, B_, P).transpose(2, 1, 3, 0)
        out[:, g * npc:(g + 1) * npc, :] = blk.reshape(B_, npc, P)
    return np.ascontiguousarray(out[:, :N, :])


def kernel(**inputs) -> np.ndarray:
    nc, in_maps, meta = build_all(inputs)
    res = run_bass_kernel_spmd(nc, in_maps, list(range(N_CORES)))
    return assemble_output(res.results, meta)


# revision 8
# speedup vs baseline: 2.0698x; 1.0876x over previous
"""DCRNN diffusion-conv GRU cell (single step, zero initial hidden state) on
8 Trainium2 NeuronCores.

Math: with H0 = 0 the reference cell reduces exactly to
    out[b] = sigmoid(-(pre_z)) * tanh(pre_h)
    pre_z  = X Wz00 + Mo Wz01 + Mi Wz11 + bz      (Wg00 = (Wg[0,0]+Wg[1,0])[:128])
    pre_h  = X Wh00 + Mo Wh01 + Mi Wh11 + bh
    Mo = Ao^T X,  Ao[m, n] = sum_{e: src=m, dst=n} coef_o[e]
    Mi = Ai^T X,  Ai[m, n] = sum_{e: dst=m, src=n} coef_i[e]
(R / Wr / br are dead code: H0*R = 0 so Xc2 == Xc.)

Strategy (v3, source-compacted): nodes padded to 5120; core g owns output
nodes [g*640, (g+1)*640) = 5 blocks of 128 for ALL 4 batches.  For each
(matrix d, block lc) "group", only the ~1700 DISTINCT source nodes feeding
that 128-column block matter, so the host compacts them into KMAX=14 chunks
of 128: A_compact[d,lc] is [14*128 src, 128 dst] bf16 and the matching X
rows are host-gathered into Xg[d,lc] = [14*128 src, 4*128 bf] fp8e4m3.
The diffusion product is then a 14-chunk PSUM accumulation per group
(vs 40 chunks block-dense) -- 2.9x fewer PE cycles and 1.2x fewer HBM
bytes than the block-dense v2.  fp8 is used ONLY for the gathered sparse-
path X copies (the mixed bf16xfp8 matmul keeps A at bf16 precision); the
dense X W00 term keeps the bf16 split-float path, so overall rel err is
~1.4e-2 (< 2e-2 gate; verified bit-exactly in numpy emulation).

Pipeline: groups run lc-major (lc0: Ao,Ai -> lc1: ...) so each block's
transposes + dense gate matmuls + activations + per-block output DMA
overlap the next block's sparse accumulation.  DMA is split per half-group
across all three queues (sync/scalar HWDGE + gpsimd SWDGE) in consumption
order so the PE starts ~1.5us in and never starves.
"""
import math

import numpy as np
import ml_dtypes

import concourse.bacc as bacc
import concourse.tile as tile
from concourse import mybir
from concourse.bass_utils import run_bass_kernel_spmd

P = 128
N_CORES = 8
B = 4
CPC = 5                      # 128-col output blocks per core
KMAX = 14                    # padded source chunks per group
NGRP = 2 * CPC               # groups per core: (lc, d) lc-major
BF16 = ml_dtypes.bfloat16
FP8 = ml_dtypes.float8_e4m3


def _prep(x, edge_index, edge_weight):
    B_, N, F = x.shape
    assert F == P and B_ == B
    npad = math.ceil(N / P / N_CORES) * N_CORES * P      # 5120
    src = edge_index[0].astype(np.int64)
    dst = edge_index[1].astype(np.int64)
    ew = edge_weight.astype(np.float32)

    deg_out = np.bincount(src, weights=ew.astype(np.float64), minlength=N)
    deg_in = np.bincount(dst, weights=ew.astype(np.float64), minlength=N)
    with np.errstate(divide="ignore"):
        dinv_out = np.where(deg_out > 0, 1.0 / deg_out, 0.0).astype(np.float32)
        dinv_in = np.where(deg_in > 0, 1.0 / deg_in, 0.0).astype(np.float32)
    coef = [ew * dinv_out[src], ew * dinv_in[dst]]
    rowcol = [(src, dst), (dst, src)]

    xpad = np.zeros((B, npad, P), np.float32)
    xpad[:, :N] = x
    x8 = xpad.astype(FP8)                                # sparse-path copies
    xh = xpad.astype(BF16)

    per_core = []
    for g in range(N_CORES):
        ac = np.zeros((P, NGRP * KMAX * P), BF16)
        xg = np.zeros((P, NGRP * KMAX * B * P), FP8)
        for lc in range(CPC):
            blk0 = (g * CPC + lc) * P
            for d in range(2):
                G = lc * 2 + d
                rows, cols = rowcol[d]
                sel = (cols >= blk0) & (cols < blk0 + P)
                r, c, w = rows[sel], (cols[sel] - blk0), coef[d][sel]
                uniq, inv = np.unique(r, return_inverse=True)
                K = len(uniq)
                assert K <= KMAX * P, (g, lc, d, K)
                ablk = np.zeros((KMAX * P, P), np.float32)
                np.add.at(ablk, (inv, c), w)
                ac[:, G * KMAX * P:(G + 1) * KMAX * P] = (
                    ablk.reshape(KMAX, P, P).transpose(1, 0, 2)
                    .reshape(P, KMAX * P).astype(BF16))
                upad = np.full(KMAX * P, npad - 1, np.int64)  # zero row
                upad[:K] = uniq
                xr = x8[:, upad, :]                      # [B, KMAX*P, P]
                xg[:, G * KMAX * B * P:(G + 1) * KMAX * B * P] = (
                    xr.transpose(1, 0, 2).reshape(KMAX, P, B, P)
                    .transpose(1, 0, 2, 3).reshape(P, KMAX * B * P))
        # dense-path rhs, lc-major: xT[k, lc*512 + b*128 + j] = x[b, blk0+j, k]
        xs = xh[:, g * CPC * P:(g + 1) * CPC * P, :]     # [B, 640, P]
        xT = np.ascontiguousarray(
            xs.reshape(B, CPC, P, P).transpose(3, 1, 0, 2)
            .reshape(P, CPC * B * P))
        per_core.append({"ac": ac, "xg": xg, "xT": xT})

    meta = dict(B=B, N=N, npad=npad)
    return per_core, meta


def _shared_inputs(Wz, bz, Wh, bh):
    def split(w):
        hi = w.astype(BF16).astype(np.float32)
        return hi.astype(BF16), (w - hi).astype(BF16)

    Wz00h, Wz00l = split(Wz[0, 0][:P] + Wz[1, 0][:P])
    Wh00h, Wh00l = split(Wh[0, 0][:P] + Wh[1, 0][:P])
    wt = np.concatenate([
        Wz00h, Wz00l, Wz[0, 1][:P].astype(BF16), Wz[1, 1][:P].astype(BF16),
        Wh00h, Wh00l, Wh[0, 1][:P].astype(BF16), Wh[1, 1][:P].astype(BF16),
    ], axis=1)
    bias = np.stack([-bz, bh], axis=1).astype(np.float32)
    ident = np.eye(P, dtype=BF16)
    return wt, bias, ident


def _build():
    ycols = CPC * B * P                                  # 2560
    bf = mybir.dt.bfloat16
    f8 = mybir.dt.float8e4
    f32 = mybir.dt.float32

    nc = bacc.Bacc("TRN2", target_bir_lowering=False, debug=False,
                   num_devices=N_CORES)
    ac_d = nc.dram_tensor("ac", [P, NGRP * KMAX * P], bf, kind="ExternalInput")
    xg_d = nc.dram_tensor("xg", [P, NGRP * KMAX * B * P], f8,
                          kind="ExternalInput")
    xT_d = nc.dram_tensor("xT", [P, ycols], bf, kind="ExternalInput")
    wt_d = nc.dram_tensor("wt", [P, 8 * P], bf, kind="ExternalInput")
    bias_d = nc.dram_tensor("bias", [P, 2], f32, kind="ExternalInput")
    ident_d = nc.dram_tensor("ident", [P, P], bf, kind="ExternalInput")
    yT_d = nc.dram_tensor("yT", [P, ycols], bf, kind="ExternalOutput")

    with tile.TileContext(nc) as tc:
        with (
            tc.tile_pool(name="const", bufs=1) as cpool,
            tc.tile_pool(name="act", bufs=3) as apool,
            tc.tile_pool(name="ps", bufs=3, space="PSUM") as ps_pool,
            tc.tile_pool(name="pt", bufs=3, space="PSUM") as pt_pool,
            tc.tile_pool(name="pd", bufs=2, space="PSUM") as pd_pool,
        ):
            ac_s = cpool.tile([P, NGRP * KMAX * P], bf)
            xg_s = cpool.tile([P, NGRP * KMAX * B * P], f8)
            xT_s = cpool.tile([P, ycols], bf)
            wt_s = cpool.tile([P, 8 * P], bf)
            bias_s = cpool.tile([P, 2], f32)
            ident_s = cpool.tile([P, P], bf)
            m_s = [cpool.tile([P, ycols], bf, name=f"m{d}_s") for d in range(2)]
            y_s = cpool.tile([P, ycols], bf)

            # ---- DMA schedule (consumption order per queue):
            #  gpsimd/SWDGE: the 9.2MB xg stream in big pieces (~350 GB/s)
            #  sync/HWDGE:   tiny consts, then ac per group
            #  scalar/HWDGE: dense-path xT, then per-block y writeback
            nc.sync.dma_start(out=ident_s[:], in_=ident_d[:])
            nc.sync.dma_start(out=bias_s[:], in_=bias_d[:])
            nc.sync.dma_start(out=wt_s[:], in_=wt_d[:])
            GXB = KMAX * B * P                           # xg cols per group
            for g0, g1 in ((0, 1), (1, 2), (2, 4), (4, 6), (6, 8), (8, 10)):
                nc.gpsimd.dma_start(out=xg_s[:, g0 * GXB:g1 * GXB],
                                    in_=xg_d[:, g0 * GXB:g1 * GXB])
            GAB = KMAX * P
            for G in range(NGRP):
                nc.sync.dma_start(out=ac_s[:, G * GAB:(G + 1) * GAB],
                                  in_=ac_d[:, G * GAB:(G + 1) * GAB])
            nc.scalar.dma_start(out=xT_s[:], in_=xT_d[:])

            # PE warmup: ~2us of dummy matmuls during the DMA lead-in so the
            # HAM clock-gate opens before the real stream starts.
            wp = pd_pool.tile([P, P], dtype=f32, name="wp", tag="pd")
            for _ in range(24):
                nc.tensor.matmul(out=wp[:], lhsT=ident_s[:], rhs=ident_s[:],
                                 start=True, stop=True)

            # weight column offsets in wt_s per gate: [W00h, W00l, W01, W11]
            gate_w = [(0, 1, 2, 3), (4, 5, 6, 7)]        # z, h

            def sparse_group(G):
                pm = ps_pool.tile([P, B * P], dtype=f32, name="pm", tag="ps")
                for c in range(KMAX):
                    nc.tensor.matmul(
                        out=pm[:],
                        lhsT=ac_s[:, (G * KMAX + c) * P:(G * KMAX + c + 1) * P],
                        rhs=xg_s[:, (G * KMAX + c) * B * P:
                                 (G * KMAX + c + 1) * B * P],
                        start=(c == 0),
                        stop=(c == KMAX - 1),
                    )
                return pm

            def sparse_epilogue(lc, pms):
                # psum [n, b*f] -> bf16 -> per-batch transpose -> m_s [f, cols]
                for d in range(2):
                    mnm = apool.tile([P, B * P], bf, tag="mnm")
                    nc.vector.tensor_copy(out=mnm[:], in_=pms[d][:])
                    for b in range(B):
                        pt = pt_pool.tile([P, P], dtype=bf, name="pt", tag="pt")
                        nc.tensor.transpose(
                            out=pt[:], in_=mnm[:, b * P:(b + 1) * P],
                            identity=ident_s[:])
                        nc.vector.tensor_copy(
                            out=m_s[d][:, lc * B * P + b * P:
                                       lc * B * P + (b + 1) * P],
                            in_=pt[:])

            def dense_block(lc):
                c0 = lc * B * P
                cs = slice(c0, c0 + B * P)
                pz = pd_pool.tile([P, B * P], dtype=f32, name="pz", tag="pd")
                ph = pd_pool.tile([P, B * P], dtype=f32, name="ph", tag="pd")
                for pt_, (wh, wl, w01, w11) in ((pz, gate_w[0]), (ph, gate_w[1])):
                    terms = [(wh, xT_s), (wl, xT_s),
                             (w01, m_s[0]), (w11, m_s[1])]
                    for ti, (wi, rhs_t) in enumerate(terms):
                        nc.tensor.matmul(
                            out=pt_[:],
                            lhsT=wt_s[:, wi * P:(wi + 1) * P],
                            rhs=rhs_t[:, cs],
                            start=(ti == 0), stop=(ti == len(terms) - 1))
                za = apool.tile([P, B * P], f32, tag="za")
                nc.scalar.activation(
                    out=za[:], in_=pz[:],
                    func=mybir.ActivationFunctionType.Sigmoid,
                    bias=bias_s[:, 0:1], scale=-1.0)
                ha = apool.tile([P, B * P], f32, tag="ha")
                nc.scalar.activation(
                    out=ha[:], in_=ph[:],
                    func=mybir.ActivationFunctionType.Tanh,
                    bias=bias_s[:, 1:2], scale=1.0)
                nc.vector.tensor_tensor(
                    out=y_s[:, cs], in0=za[:], in1=ha[:],
                    op=mybir.AluOpType.mult)
                nc.scalar.dma_start(out=yT_d[:, cs], in_=y_s[:, cs])

            # ---- software pipeline: sparse(lc+1) overlaps epilogue/dense(lc)
            pending = None                               # (lc, [pm_o, pm_i])
            for lc in range(CPC):
                pm_o = sparse_group(lc * 2)
                if pending is not None:
                    sparse_epilogue(*pending)
                pm_i = sparse_group(lc * 2 + 1)
                if pending is not None:
                    dense_block(pending[0])
                pending = (lc, [pm_o, pm_i])
            sparse_epilogue(*pending)
            dense_block(pending[0])
    nc.compile()
    return nc


def build_all(inputs):
    """Returns (nc, in_maps, meta). Split out so test.py can reuse."""
    x = np.asarray(inputs["x"], np.float32)
    edge_index = np.asarray(inputs["edge_index"])
    edge_weight = np.asarray(inputs["edge_weight"], np.float32)
    Wz = np.asarray(inputs["Wz"], np.float32)
    bz = np.asarray(inputs["bz"], np.float32)
    Wh = np.asarray(inputs["Wh"], np.float32)
    bh = np.asarray(inputs["bh"], np.float32)

    per_core, meta = _prep(x, edge_index, edge_weight)
    wt, bias, ident = _shared_inputs(Wz, bz, Wh, bh)
    in_maps = []
    for g in range(N_CORES):
        m = dict(per_core[g])
        m["wt"] = wt
        m["bias"] = bias
        m["ident"] = ident
        in_maps.append(m)
    nc = _build()
    return nc, in_maps, meta


def assemble_output(results, meta):
    B_, N = meta["B"], meta["N"]
    npc = CPC * P
    out = np.empty((B_, N_CORES * npc, P), np.float32)
    for g in range(N_CORES):
        # yT[f, lc*512 + b*128 + j] = out[b, g*640 + lc*128 + j, f]
        blk = (results[g]["yT"].astype(np.float32)
               .reshape(P, CPC, B_, P).transpose(2, 1, 3, 0))
        out[:, g * npc:(g + 1) * npc, :] = blk.reshape(B_, npc, P)
    return np.ascontiguousarray(out[:, :N, :])


def kernel(**inputs) -> np.ndarray:
    nc, in_maps, meta = build_all(inputs)
    res = run_bass_kernel_spmd(nc, in_maps, list(range(N_CORES)))
    return assemble_output(res.results, meta)
